# revision 8
# baseline (speedup 1.0000x reference)
"""Distributed Trainium2 kernel for nn_AnchorLoss.

loss = sum_{b,i,j : mask[b,i,j]==1} (1 - exp(-||pos_i - pos_j||^2 / T)),  pos = embedding + abs_coords

Strategy (8 NeuronCores, batch b -> core b):
  loss = count(mask==1) - S,   S = sum_{mask==1} exp(-sq_dist / T)
  Per core, sq_dist is produced on the TensorEngine as a K=4 matmul:
      Q[i] = [x_i, y_i, r_i, 1],  K[j] = [-2x_j, -2y_j, 1, r_j],  Q.K = |p_i - p_j|^2
  A second identity-matmul accumulates a host-built penalty (0 where mask==1,
  BIG elsewhere) into the same PSUM tile, so a single ScalarE
  exp(-x/T) pass with accum_out computes the masked row-sums directly
  (exp of the penalized entries underflows to 0).
  Host sums the per-core [128, NT] accumulators in float64.
"""

import numpy as np

B, N, D = 8, 2048, 2
TEMPERATURE = 10.0
P = 128
NT = N // P          # 16 row tiles
CHUNK = 512          # matmul free-dim (one PSUM bank of f32)
BIG = 1536.0         # exp(-BIG/T) == 0 in f32

TRACE = False        # set True (e.g. from test.py) to neuron-profile the run
LAST_RESULTS = None  # BassKernelResults of the last run when TRACE

_cache = {}


def _build():
    from concourse import bacc, mybir
    from concourse.tile import TileContext

    nc = bacc.Bacc()
    f32 = mybir.dt.float32
    f16 = mybir.dt.float16
    # consts packed into one tensor -> one DMA -> one wait semaphore
    # (a matmul waiting on >1 DMA queue sem trips a walrus LDWEIGHTS limit)
    consts = nc.declare_dram_parameter("consts", [P, P + 2 * N], f16, isOutput=False)
    pen = nc.declare_dram_parameter("pen", [N, N], f16, isOutput=False)
    out = nc.declare_dram_parameter("out", [P, NT], f32, isOutput=True)

    with TileContext(nc) as tc:
        with (
            tc.tile_pool(name="singles", bufs=1) as singles,
            tc.tile_pool(name="pens", bufs=3) as pens,
            tc.tile_pool(name="psum", bufs=2, space="PSUM") as psum_pool,
        ):
            c_sb = singles.tile([P, P + 2 * N], f16)
            acc = singles.tile([P, NT], f32)
            nc.sync.dma_start(out=c_sb, in_=consts[:, :])
            id_sb = c_sb[:, 0:P]
            qt_sb = c_sb[0:4, P:P + N]
            kt_sb = c_sb[0:4, P + N:P + 2 * N]
            for t in range(NT):
                pen_t = pens.tile([P, N], f16)
                nc.sync.dma_start(out=pen_t, in_=pen[t * P:(t + 1) * P, :])
                ps = psum_pool.tile([P, N], f32)
                for c in range(N // CHUNK):
                    sl = slice(c * CHUNK, (c + 1) * CHUNK)
                    nc.tensor.matmul(
                        ps[:, sl], lhsT=qt_sb[:, t * P:(t + 1) * P],
                        rhs=kt_sb[:, sl], start=True, stop=False,
                    )
                    nc.tensor.matmul(
                        ps[:, sl], lhsT=id_sb[:, :],
                        rhs=pen_t[:, sl], start=False, stop=True,
                    )
                nc.scalar.activation(
                    out=ps[:, :], in_=ps[:, :],
                    func=mybir.ActivationFunctionType.Exp,
                    scale=-1.0 / TEMPERATURE,
                    accum_out=acc[:, t:t + 1],
                )
            nc.sync.dma_start(out=out[:, :], in_=acc[:, :])
    nc.compile()
    return nc


def kernel(embedding, abs_coords, patch_mask):
    global LAST_RESULTS
    from concourse.bass_utils import run_bass_kernel_spmd

    if "nc" not in _cache:
        _cache["nc"] = _build()
    nc = _cache["nc"]

    pos = embedding.astype(np.float64) + abs_coords.astype(np.float64)  # [B,N,D]
    x = pos[:, :, 0]
    y = pos[:, :, 1]
    r = x * x + y * y
    ones = np.ones_like(x)
    qt_all = np.stack([x, y, r, ones], axis=1).astype(np.float32)          # [B,4,N]
    kt_all = np.stack([-2.0 * x, -2.0 * y, ones, r], axis=1).astype(np.float32)
    ident = np.eye(P, dtype=np.float32)

    in_maps = []
    for b in range(B):
        consts_b = np.zeros((P, P + 2 * N), dtype=np.float16)
        consts_b[:, 0:P] = ident
        consts_b[0:4, P:P + N] = qt_all[b]
        consts_b[0:4, P + N:P + 2 * N] = kt_all[b]
        pen_b = np.where(patch_mask[b] == 1, 0.0, BIG).astype(np.float16)
        in_maps.append({"consts": consts_b, "pen": pen_b})

    res = run_bass_kernel_spmd(
        nc, in_maps, core_ids=list(range(B)),
        trace=TRACE, trace_cores=[0] if TRACE else None,
    )
    LAST_RESULTS = res

    s_hw = sum(res.results[b]["out"].astype(np.float64).sum() for b in range(B))
    count = np.count_nonzero(patch_mask == 1)
    loss = np.float64(count) - s_hw
    return np.array(loss, dtype=np.float32)


# revision 9
# speedup vs baseline: 1.9160x; 1.9160x over previous
"""Distributed Trainium2 kernel for nn_AnchorLoss.

loss = sum_{b,i,j : mask[b,i,j]==1} (1 - exp(-||pos_i - pos_j||^2 / T)),  pos = embedding + abs_coords

Strategy (8 NeuronCores, batch b -> core b):
  loss = count(mask==1) - diag(mask) - 2 * S,   S = sum_{i<j} (msum_ij/2) exp(-sq_ij/T)
  with msum = mask + mask^T (exp is symmetric in (i,j), so only the upper
  triangle is computed; the diagonal has exp(0)=1 and is handled on host).

  Per core the whole per-tile computation is ONE TensorEngine pass:
  a K=(4+M) contraction where rows 0-3 compute sq_dist via
      Q[i] = [x_i, y_i, r_i, 1],  K[j] = [-2x_j, -2y_j, 1, r_j]
  and rows 4..4+M are an identity that accumulates a host-built penalty
      p = -T*ln(msum/2)  ->  {0, T*ln2, BIG}
  into the same PSUM. A single ScalarE exp(-x/T) with accum_out then yields
  the weighted masked row-sums directly (exp(-BIG/T) underflows to 0).
  Host sums the per-core accumulators in float64.
"""

import numpy as np

B, N, D = 8, 2048, 2
TEMPERATURE = 10.0
P = 128
MT = 124                      # rows per tile (K = 4 + MT <= 128)
NTILES = (N + MT - 1) // MT   # 17 (last tile has 64 rows)
CHUNK = 512                   # matmul free-dim (one PSUM bank of f32)
BIG = 1536.0                  # exp(-BIG/T) == 0 in f32
LN2T = float(TEMPERATURE * np.log(2.0))  # penalty for msum==1 (weight 1/2)

TRACE = False        # set True (e.g. from test.py) to neuron-profile the run
LAST_RESULTS = None  # BassKernelResults of the last run when TRACE

_cache = {}


def _tiles():
    for k in range(NTILES):
        i0 = k * MT
        m = min(MT, N - i0)     # rows in this tile
        fd = N - i0             # columns processed (upper triangle)
        yield k, i0, m, fd


def _build():
    from concourse import bacc, mybir
    from concourse.tile import TileContext

    nc = bacc.Bacc()
    f32 = mybir.dt.float32
    f16 = mybir.dt.float16
    stat = nc.declare_dram_parameter("stat", [P, N], f16, isOutput=False)
    mov = nc.declare_dram_parameter("mov", [NTILES * P, N], f16, isOutput=False)
    out = nc.declare_dram_parameter("out", [P, NTILES], f32, isOutput=True)

    with TileContext(nc) as tc:
        with (
            tc.tile_pool(name="singles", bufs=1) as singles,
            tc.tile_pool(name="movs", bufs=3) as movs,
            tc.tile_pool(name="psum", bufs=2, space="PSUM") as psum_pool,
        ):
            stat_sb = singles.tile([P, N], f16)
            acc = singles.tile([P, NTILES], f32)
            nc.vector.memset(acc, 0.0)
            nc.sync.dma_start(out=stat_sb, in_=stat[:, :])
            for k, i0, m, fd in _tiles():
                kk = 4 + m  # contraction size
                mv = movs.tile([P, N], f16, tag="mv")
                nc.sync.dma_start(
                    out=mv[0:kk, 0:fd], in_=mov[k * P:k * P + kk, i0:N]
                )
                ps = psum_pool.tile([P, N], f32, tag="ps")
                for c0 in range(0, fd, CHUNK):
                    c1 = min(c0 + CHUNK, fd)
                    nc.tensor.matmul(
                        ps[0:m, c0:c1],
                        lhsT=stat_sb[0:kk, i0:i0 + m],
                        rhs=mv[0:kk, c0:c1],
                        start=True, stop=True,
                    )
                nc.scalar.activation(
                    out=ps[0:m, 0:fd], in_=ps[0:m, 0:fd],
                    func=mybir.ActivationFunctionType.Exp,
                    scale=-1.0 / TEMPERATURE,
                    accum_out=acc[0:m, k:k + 1],
                )
            nc.sync.dma_start(out=out[:, :], in_=acc[:, :])
    nc.compile()
    return nc


_TRIU = None


def _host_prep(embedding, abs_coords, patch_mask):
    global _TRIU
    if _TRIU is None:
        _TRIU = np.triu(np.ones((N, N), dtype=bool), k=1)

    pos = embedding.astype(np.float64) + abs_coords.astype(np.float64)  # [B,N,D]
    x = pos[:, :, 0]
    y = pos[:, :, 1]
    r = x * x + y * y
    ones = np.ones_like(x)
    qt_all = np.stack([x, y, r, ones], axis=1).astype(np.float16)          # [B,4,N]
    kt_all = np.stack([-2.0 * x, -2.0 * y, ones, r], axis=1).astype(np.float16)

    in_maps = []
    for b in range(B):
        mb = (patch_mask[b] == 1)
        msum = mb.astype(np.int8) + mb.astype(np.int8).T
        pen = np.where(msum == 2, 0.0, np.where(msum == 1, LN2T, BIG))
        pen = np.where(_TRIU, pen, BIG).astype(np.float16)

        stat_b = np.zeros((P, N), dtype=np.float16)
        mov_b = np.zeros((NTILES * P, N), dtype=np.float16)
        for k, i0, m, fd in _tiles():
            stat_b[0:4, i0:i0 + m] = qt_all[b][:, i0:i0 + m]
            stat_b[4:4 + m, i0:i0 + m] = np.eye(m, dtype=np.float16)
            mov_b[k * P:k * P + 4, :] = kt_all[b]
            mov_b[k * P + 4:k * P + 4 + m, :] = pen[i0:i0 + m, :]
        in_maps.append({"stat": stat_b, "mov": mov_b})
    return in_maps


def kernel(embedding, abs_coords, patch_mask):
    global LAST_RESULTS
    from concourse.bass_utils import run_bass_kernel_spmd

    if "nc" not in _cache:
        _cache["nc"] = _build()
    nc = _cache["nc"]

    in_maps = _host_prep(embedding, abs_coords, patch_mask)

    res = run_bass_kernel_spmd(
        nc, in_maps, core_ids=list(range(B)),
        trace=TRACE, trace_cores=[0] if TRACE else None,
    )
    LAST_RESULTS = res

    s_hw = sum(res.results[b]["out"].astype(np.float64).sum() for b in range(B))
    count = np.count_nonzero(patch_mask == 1)
    diag_cnt = sum(int(np.trace((patch_mask[b] == 1).astype(np.int64))) for b in range(B))
    loss = np.float64(count) - 2.0 * s_hw - np.float64(diag_cnt)
    return np.array(loss, dtype=np.float32)


# revision 12
# speedup vs baseline: 2.3028x; 1.2019x over previous
"""Distributed Trainium2 (Bass) kernel for nn_AnchorLoss.

Reference:
  pos  = embedding + abs_coords                     [B, N, D],  B=8, N=2048, D=2
  sq   = ||pos_i - pos_j||^2                        [B, N, N]
  loss = sum over (b,i,j) with patch_mask==1 of (1 - exp(-sq / T))

Distribution: batch b -> NeuronCore b (8 cores, data parallel). Each core
computes a partial sum; the host combines them (the all-reduce of a scalar
is free host-side since kernel() returns the full output anyway).

Math (per core):
  loss = count(mask==1) - diag(mask) - 2 * S
  S    = sum_{i<j} (msum_ij / 2) * exp(-sq_ij / T),   msum = mask + mask^T
  (exp term is symmetric in (i,j) so only the upper triangle is computed;
   diagonal terms have exp(0)=1 and cancel exactly on host.)

Kernel (per core) — the entire per-tile computation is ONE TensorEngine pass:
  The triangle is row-tiled into NTILES tiles of MT=124 rows; tile k covers
  rows [124k, 124k+m) x cols [124k, N). A K=(4+m) contraction computes
    rows 0-3:    sq via  Q[i]=[x_i,y_i,r_i,1] . K[j]=[-2x_j,-2y_j,1,r_j]
    rows 4-4+m:  an identity that accumulates a host-built penalty
                 p = -T*ln(msum/2) in {0, T*ln2, BIG}  into the same PSUM
  so PSUM holds sq + p. A ScalarE exp(-x/T) with accum_out then yields the
  weighted row-sums directly: weight exp(-p/T) is {1, 1/2, 0} (exp(-BIG/T)
  underflows to exactly 0, which also implements the triangle masking).
  Output rows m..127 are forced to BIG through dummy stationary columns
  [0,0,BIG,0] (K row 2 is all-ones) so every PSUM row is defined and exps
  to 0 — this lets several small tiles share one PSUM half and one ACTIVATE
  (the reduction is a grand sum, so mixing tiles in one accumulator is fine).
  Tiles run smallest-first (DMA size ramps with PE consumption); small tiles
  are binned 3/3/2 so the ScalarE queue carries 12 ACTIVATEs instead of 17.
  fp16 operands (penalties and the identity are fp16-exact; fp16 matmul runs
  the PE at full rate, unlike fp32 which is 4x slower).

  Per tile, the [K, 128] stationary block and the [K, fd] moving block are
  packed side by side in one DRAM row-block -> a single DMA per tile.
  Hand-rolled pipeline (raw bacc, no TileContext):
    sync:   DMA tile into one of NSLOT sbuf slots
    tensor: fused matmul into one of 2 PSUM halves (512-col bank chunks)
    scalar: in-place exp over the bin + accumulator read into acc[:, bin]
  Host sums the per-core [128, NBINS] accumulators in float64.
"""

from contextlib import ExitStack

import numpy as np

B, N, D = 8, 2048, 2
TEMPERATURE = 10.0
P = 128
MT = 124                      # rows per tile (K = 4 + MT <= 128)
NTILES = (N + MT - 1) // MT   # 17 (last tile has 64 rows)
CHUNK = 512                   # PSUM bank width in f32
BIG = 1536.0                  # exp(-BIG/T) == 0 in f32
LN2T = float(TEMPERATURE * np.log(2.0))  # penalty giving weight 1/2
MOVW = P + N                  # stat block (128 cols incl dummies) + moving cols
NSLOT = 6                     # mv buffers (DMA prefetch depth)
# bins preserve the descending tile order; each bin fits one 2048-col PSUM half
BINS = [[16, 15, 14], [13, 12, 11], [10, 9], [8], [7], [6], [5], [4], [3], [2], [1], [0]]

TRACE = False        # set True (see test.py) to neuron-profile the run
LAST_RESULTS = None  # BassKernelResults of the last run when TRACE

_cache = {}


def _tile_geom(k):
    i0 = k * MT
    m = min(MT, N - i0)
    fd = N - i0
    return i0, m, fd


def _build():
    from concourse import bacc, mybir

    nc = bacc.Bacc(enable_partition_id=False)
    f32 = mybir.dt.float32
    f16 = mybir.dt.float16
    mov = nc.declare_dram_parameter("mov", [NTILES * P, MOVW], f16, isOutput=False)
    out = nc.declare_dram_parameter("out", [P, len(BINS)], f32, isOutput=True)

    seq = []   # (tile_k, bin_idx, col_off)
    for bi, tks in enumerate(BINS):
        off = 0
        for k in tks:
            seq.append((k, bi, off))
            off += _tile_geom(k)[2]
        assert off <= N
    tiles_through_bin = {}
    cnt = 0
    for bi, tks in enumerate(BINS):
        cnt += len(tks)
        tiles_through_bin[bi] = cnt

    with ExitStack() as ctx:
        mvs = [
            ctx.enter_context(nc.sbuf_tensor(f"mv{j}", [P, MOVW], f16))
            for j in range(NSLOT)
        ]
        acc = ctx.enter_context(nc.sbuf_tensor("acc", [P, len(BINS)], f32))
        pss = [
            ctx.enter_context(nc.psum_tensor(f"ps{j}", [P, N], f32)) for j in range(2)
        ]
        dma_sems = [
            ctx.enter_context(nc.semaphore(f"dma{j}")) for j in range(NSLOT)
        ]
        pe_sem = ctx.enter_context(nc.semaphore("pe"))
        act_sem = ctx.enter_context(nc.semaphore("act"))
        odma_sem = ctx.enter_context(nc.semaphore("odma"))
        block = ctx.enter_context(nc.Block())

        @block.sync
        def _(sync):
            for s, (k, bi, off) in enumerate(seq):
                i0, m, fd = _tile_geom(k)
                kk = 4 + m
                if s >= NSLOT:
                    # slot reuse: tile s-NSLOT must be consumed by PE first
                    sync.wait_ge(pe_sem, s - NSLOT + 1)
                sync.dma_start(
                    out=mvs[s % NSLOT][0:kk, 0:P + fd],
                    in_=mov[k * P:k * P + kk, 0:P + fd],
                ).then_inc(dma_sems[s % NSLOT], 16)

        @block.tensor
        def _(tensor):
            for s, (k, bi, off) in enumerate(seq):
                i0, m, fd = _tile_geom(k)
                kk = 4 + m
                mv = mvs[s % NSLOT]
                ps = pss[bi % 2]
                tensor.wait_ge(dma_sems[s % NSLOT], 16 * (s // NSLOT + 1))
                if off == 0 and bi >= 2:
                    # PSUM half ping-pong: exp of bin bi-2 must have read it
                    tensor.wait_ge(act_sem, bi - 1)
                # chunk on absolute psum columns, split at 512 bank boundaries
                c0 = off
                while c0 < off + fd:
                    c1 = min(off + fd, (c0 // CHUNK + 1) * CHUNK)
                    mm = tensor.matmul(
                        ps[0:P, c0:c1],
                        lhsT=mv[0:kk, 0:P],
                        rhs=mv[0:kk, P + (c0 - off):P + (c1 - off)],
                        start=True, stop=True,
                    )
                    c0 = c1
                mm.then_inc(pe_sem, 1)

        @block.scalar
        def _(scalar):
            for bi, tks in enumerate(BINS):
                binw = sum(_tile_geom(k)[2] for k in tks)
                ps = pss[bi % 2]
                scalar.wait_ge(pe_sem, tiles_through_bin[bi])
                scalar.activation(
                    out=ps[0:P, 0:binw], in_=ps[0:P, 0:binw],
                    func=mybir.ActivationFunctionType.Exp,
                    scale=-1.0 / TEMPERATURE,
                    accum_out=acc[0:P, bi:bi + 1],
                ).then_inc(act_sem, 1)
            # act_sem increments at instruction *completion*; without this wait
            # the DMA could read acc before the last accum write lands in SBUF
            scalar.wait_ge(act_sem, len(BINS))
            scalar.dma_start(out=out[:, :], in_=acc[:, :]).then_inc(odma_sem, 16)
            scalar.wait_ge(odma_sem, 16)

    nc.compile()
    return nc


_TRIU = None


def _host_prep(embedding, abs_coords, patch_mask):
    global _TRIU
    if _TRIU is None:
        _TRIU = np.triu(np.ones((N, N), dtype=bool), k=1)

    pos = embedding.astype(np.float64) + abs_coords.astype(np.float64)  # [B,N,D]
    x = pos[:, :, 0]
    y = pos[:, :, 1]
    r = x * x + y * y
    ones = np.ones_like(x)
    qt_all = np.stack([x, y, r, ones], axis=1).astype(np.float16)          # [B,4,N]
    kt_all = np.stack([-2.0 * x, -2.0 * y, ones, r], axis=1).astype(np.float16)

    eye = np.eye(MT, dtype=np.float16)
    in_maps = []
    for b in range(B):
        mb = patch_mask[b] == 1
        msum = mb.astype(np.int8) + mb.astype(np.int8).T
        pen = np.where(msum == 2, 0.0, np.where(msum == 1, LN2T, BIG))
        pen = np.where(_TRIU, pen, BIG).astype(np.float16)

        mov_b = np.zeros((NTILES * P, MOVW), dtype=np.float16)
        for k in range(NTILES):
            i0, m, fd = _tile_geom(k)
            blk = mov_b[k * P:k * P + 4 + m]
            blk[0:4, 0:m] = qt_all[b][:, i0:i0 + m]          # stationary: Q
            blk[4:4 + m, 0:m] = eye[0:m, 0:m]                # stationary: identity
            # dummy output rows m..127: [0,0,BIG,0] . [.,.,1,.] = BIG -> exp 0
            blk[2, m:P] = BIG
            blk[0:4, P:P + fd] = kt_all[b][:, i0:N]          # moving: K
            blk[4:4 + m, P:P + fd] = pen[i0:i0 + m, i0:N]    # moving: penalties
        in_maps.append({"mov": mov_b})
    return in_maps


def kernel(embedding, abs_coords, patch_mask):
    global LAST_RESULTS
    from concourse.bass_utils import run_bass_kernel_spmd

    embedding = np.asarray(embedding)
    abs_coords = np.asarray(abs_coords)
    patch_mask = np.asarray(patch_mask)

    if "nc" not in _cache:
        _cache["nc"] = _build()
    nc = _cache["nc"]

    in_maps = _host_prep(embedding, abs_coords, patch_mask)

    res = run_bass_kernel_spmd(
        nc, in_maps, core_ids=list(range(B)),
        trace=TRACE, trace_cores=[0] if TRACE else None,
    )
    LAST_RESULTS = res

    s_hw = sum(res.results[b]["out"].astype(np.float64).sum() for b in range(B))
    count = np.count_nonzero(patch_mask == 1)
    diag_cnt = sum(
        int(np.trace((patch_mask[b] == 1).astype(np.int64))) for b in range(B)
    )
    loss = np.float64(count) - 2.0 * s_hw - np.float64(diag_cnt)
    return np.array(loss, dtype=np.float32)
